# revision 30
# baseline (speedup 1.0000x reference)
"""Trainium2 Bass kernel for CyclicShiftConv (Hilbert-rotation SE attention).

out[b,c,l] = sum_r softmax_r(MLP(mean_l x[b,c,rot_idx[r,l]]))[b,c,r] * x[b,c,rot_idx[r,l]]

Strategy (8 cores, data-parallel over batch; 2 samples = 512 (b,c) rows/core).
The kernel is DMA-bound (360 GB/s shared DMA fabric), so the design minimizes
bytes moved:
  - x (8 MiB f32/core) is loaded once, PE-transposed into an fp16 x^T resident
    in SBUF (no DRAM round-trip for the transposed copy).
  - mean over l of the gathered tensor == x^T contracted with bincount/L, so
    the SE-MLP input comes from tiny PE matmuls against an fp16 count vector.
  - The rotation gathers run SBUF->SBUF with the fp16 transpose-mode
    dma_gather (tokens_per_rank=128 maps token l' to partition l'%128 / rank
    l'//128, exactly the x^T stripe layout).  Transpose-mode writes return the
    data in natural [bc, l] layout, so no transpose-back matmuls are needed.
    fp16 halves gather bytes vs f32.
  - rot_idx[0] is the identity for the Hilbert rotation set; when the host
    detects this, rotation 0 is not gathered at all - its contribution is
    taken from the natural-layout f32 x directly (exact, and 4 MiB less DMA).
  - The softmax-weighted sum over rotations runs on the PE as matmuls against
    diagonal weight matrices accumulating in PSUM (out_j = sum_r
    diag(W_r) @ g_r), keeping the elementwise engines free.
"""

import sys

for _p in ("/opt/trn_rl_repo", "/opt/pypackages"):
    if _p not in sys.path:
        sys.path.append(_p)

import numpy as np

B, C, L, R, RED = 16, 256, 4096, 4, 16
NCORES = 8
BPC = B // NCORES          # samples per core
BC = BPC * C               # 512 rows per core
NT = L // 128              # 32 transpose stripes
NCH = 256                  # phase-2 max l-positions per processing chunk
# gather calls (col0, width): big early for DMA density / low desc-gen
# overhead, small late so the post-last-gather tail is short
GATHERS = (
    [(i * 512, 512) for i in range(5)]
    + [(2560 + i * 256, 256) for i in range(5)]
    + [(3840, 128), (3968, 128)]
)
# processing chunks (col0, width): PE/drain/store granularity
PCHUNKS = [(i * 256, 256) for i in range(15)] + [(3840, 128), (3968, 128)]
GBUFS = {512: 2, 256: 3, 128: 2}   # gather-buffer ring depth per width

# f32 constant block: [128, CST_W]
CST_ID = 0                 # identity [128, 128]
CST_W1 = 128               # w1       [128, 32]
CST_B2 = 160               # b2       [128, 2]
CST_W2 = 162               # w2       [16, 256] (partitions 0:16)
CST_B1 = 418               # b1       [16, 1]
CST_W = 419

# fp16 constant block: [128, 256]
C16_ID = 0                 # identity [128, 128]
C16_CNT = 128              # cnt      [128, 128]: cnt16[p, 4t+r] = cnt[r, t*128+p]
C16_W = 256

_NC_CACHE = {}


def _build_nc(skip_r0, loop_n=1):
    import concourse.bass as bass
    import concourse.mybir as mybir
    from concourse import bacc
    from concourse.tile import TileContext
    from contextlib import ExitStack

    f32 = mybir.dt.float32
    f16 = mybir.dt.float16
    i16 = mybir.dt.int16
    AF = mybir.ActivationFunctionType
    AX = mybir.AxisListType
    ALU = mybir.AluOpType

    NRG = 3 if skip_r0 else 4      # rotations gathered
    NIDXC = NRG * L // 16          # idx table columns

    f32r = mybir.dt.float32r

    nc = bacc.Bacc(
        "TRN2",
        target_bir_lowering=False,
        debug=False,
        enable_asserts=False,
        num_devices=NCORES,
    )

    x_in = nc.dram_tensor("x", [BC, L], f32, kind="ExternalInput").ap()
    cst_in = nc.dram_tensor("cst", [128, CST_W], f32, kind="ExternalInput").ap()
    cst16_in = nc.dram_tensor("cst16", [128, C16_W], f16, kind="ExternalInput").ap()
    idx_in = nc.dram_tensor("idx", [128, NIDXC], i16, kind="ExternalInput").ap()
    out = nc.dram_tensor("out", [BC, L], f16, kind="ExternalOutput").ap()

    x3 = x_in.rearrange("(j p) l -> p j l", j=4)
    out3 = out.rearrange("(j p) l -> p j l", j=4)

    with TileContext(nc) as tc, ExitStack() as ctx:
        if loop_n > 1:
            ctx.enter_context(tc.For_i(0, loop_n, 1))
        cpool = ctx.enter_context(tc.tile_pool(name="consts", bufs=1))
        xpool = ctx.enter_context(tc.tile_pool(name="xp", bufs=1))
        # phase-2 pools opened up-front so their SBUF ranges never overlap
        # the phase-1/MLP pools (address reuse creates false WAR waits)
        gpool = ctx.enter_context(tc.tile_pool(name="gp", bufs=6))
        opool = ctx.enter_context(tc.tile_pool(name="op", bufs=3))

        cst = cpool.tile([128, CST_W], f32, name="cst")
        nc.sync.dma_start(cst[:], cst_in)
        cst16 = cpool.tile([128, C16_W], f16, name="cst16")
        idx_t = cpool.tile([128, NIDXC], i16, name="idx_t")

        ident = cst[:, CST_ID : CST_ID + 128]
        w1_t = cst[:, CST_W1 : CST_W1 + 32]
        b2_t = cst[:, CST_B2 : CST_B2 + 2]
        w2_t = cst[0:16, CST_W2 : CST_W2 + 256]
        b1_t = cst[0:16, CST_B1 : CST_B1 + 1]
        ident16 = cst16[:, C16_ID : C16_ID + 128]
        cnt16 = cst16[:, C16_CNT : C16_CNT + 128]

        # x natural fp16: [128, j, L], bc = j*128 + p (kept through phase 2)
        x16 = xpool.tile([128, 4, L], f16, name="x16")
        # x^T fp16: xt[p, t*512 + k] = x[bc=k, l=t*128+p]
        xt = xpool.tile([128, NT * BC], f16, name="xt")
        s_sb = cpool.tile([4, BC], f32, name="s_sb")
        # diagonal softmax-weight matrices for the PE weighted sum
        dg16 = cpool.tile([128, 4 * 4 * 128], f16, name="dg16")

        # ---------------- phase 1: load, transpose -> xt fp16, s = x @ cnt --
        # late-half x16 casts deferred past the MLP: (engine-parity, src AP,
        # dst AP) emitted after the softmax so the cast queue never delays it
        late_casts = []
        with (
            tc.tile_pool(name="xsp", bufs=6) as xspool,
            tc.tile_pool(name="pp1", bufs=4, space="PSUM") as pp1,
            tc.tile_pool(name="pps", bufs=1, space="PSUM") as pps,
        ):
            # shrinking load chunks: the final small chunks minimize the
            # last-stripe transpose+drain latency that gates the first gather
            widths = [512] * 7 + [256, 256]
            col = 0
            tglob = 0
            for h, w in enumerate(widths):
                if h >= 7:
                    # final chunks live in the persistent pool: their x16
                    # casts are deferred past the MLP (see late_casts)
                    xh = xpool.tile([128, 4, w], f32, name=f"xh_l{h}")
                else:
                    xh = xspool.tile([128, 4, w], f32, name="xh")
                nc.sync.dma_start(xh[:], x3[:, :, col : col + w])
                if h == 0:
                    # constants not needed until mid-phase; keep x loads first
                    nc.sync.dma_start(cst16[:], cst16_in)
                    nc.sync.dma_start(idx_t[:], idx_in)
                for tt in range(w // 128):
                    t = tglob + tt
                    pt = pp1.tile([128, BC], f32, name="pt")
                    for j in range(4):
                        # fp32r transpose (1.5 cyc/row vs 2): >=16 mantissa
                        # bits survive, and xt is rounded to fp16 anyway
                        nc.tensor.transpose(
                            pt[:, j * 128 : (j + 1) * 128].bitcast(f32r),
                            xh[:, j, tt * 128 : (tt + 1) * 128].bitcast(f32r),
                            ident.bitcast(f32r),
                        )
                    xt_t = xt[:, t * BC : (t + 1) * BC]
                    if t % 2 == 0:
                        nc.vector.tensor_copy(xt_t, pt[:])
                    else:
                        nc.scalar.copy(xt_t, pt[:])
                # natural fp16 copy (consumed by phase 2's rotation-0 term);
                # NCH-aligned pieces, emitted after the drains so they never
                # head-of-line-block an xt drain in the engine queues
                for piece in range(w // NCH):
                    cast_src = xh[:, :, piece * NCH : (piece + 1) * NCH]
                    cast_dst = x16[:, :, col + piece * NCH : col + (piece + 1) * NCH]
                    if h < 7:
                        if (h + piece) % 2 == 0:
                            nc.vector.tensor_copy(cast_dst, cast_src)
                        else:
                            nc.scalar.copy(cast_dst, cast_src)
                    else:
                        late_casts.append(((h + piece) % 2, cast_src, cast_dst))
                col += w
                tglob += w // 128
            # batched mean matmuls (decoupled from the per-stripe chain so the
            # PE never stalls on a drain mid-phase)
            psum_s = pps.tile([4, BC], f32, name="psum_s")
            for t in range(NT):
                nc.tensor.matmul(
                    psum_s[:],
                    cnt16[:, 4 * t : 4 * t + 4],
                    xt[:, t * BC : (t + 1) * BC],
                    start=(t == 0),
                    stop=(t == NT - 1),
                )
            nc.vector.tensor_copy(s_sb[:], psum_s[:])

        # ---------------- SE MLP + softmax over rotations -------------------
        with (
            tc.tile_pool(name="mlp", bufs=1) as mpool,
            tc.tile_pool(name="ppm", bufs=1, space="PSUM") as ppm,
        ):
            sT = []
            for j in range(4):
                p_sT = ppm.tile([128, 4], f32, name="p_sT")
                nc.tensor.transpose(
                    p_sT[:], s_sb[:, j * 128 : (j + 1) * 128], cst[0:4, CST_ID : CST_ID + 4]
                )
                sTj = mpool.tile([128, 4], f32, name=f"sT{j}")
                nc.vector.tensor_copy(sTj[:], p_sT[:])
                sT.append(sTj)
            hs = []
            for b in range(BPC):
                p_h = ppm.tile([16, 4], f32, name="p_h")
                for hi in range(2):
                    nc.tensor.matmul(
                        p_h[:],
                        w1_t[:, hi * 16 : (hi + 1) * 16],
                        sT[2 * b + hi][:],
                        start=(hi == 0),
                        stop=(hi == 1),
                    )
                h_sb = mpool.tile([16, 4], f32, name=f"h{b}")
                nc.scalar.activation(h_sb[:], p_h[:], AF.Relu, bias=b1_t)
                hs.append(h_sb)
            p_sc = ppm.tile([128, 16], f32, name="p_sc")
            for b in range(BPC):
                for hi in range(2):
                    j = 2 * b + hi
                    nc.tensor.matmul(
                        p_sc[:, 4 * j : 4 * j + 4],
                        w2_t[:, hi * 128 : (hi + 1) * 128],
                        hs[b][:],
                        start=True, stop=True,
                    )
            sc_all = mpool.tile([128, 4, 4], f32, name="sc_all")
            b2ap = b2_t
            b2v = bass.AP(
                b2ap.tensor, b2ap.offset, [b2ap.ap[0], [0, 2], b2ap.ap[1], [0, 4]]
            )
            nc.vector.tensor_tensor(
                sc_all[:].rearrange("p (b hi) r -> p b hi r", b=2),
                p_sc[:].rearrange("p (b hi r) -> p b hi r", b=2, hi=2),
                b2v,
                op=ALU.add,
            )
            # no max-subtraction: scores are mean-pooled SE-MLP outputs with
            # |score| << 80, so exp cannot overflow in f32
            e_all = mpool.tile([128, 4, 4], f32, name="e_all")
            nc.scalar.activation(
                e_all[:].rearrange("p a r -> p (a r)"),
                sc_all[:].rearrange("p a r -> p (a r)"),
                AF.Exp,
            )
            sm = mpool.tile([128, 4], f32, name="sm")
            nc.vector.reduce_sum(sm[:], e_all[:], axis=AX.X)
            rcp = mpool.tile([128, 4], f32, name="rcp")
            nc.vector.reciprocal(rcp[:], sm[:])
            rc = rcp[:]
            rcv = bass.AP(rc.tensor, rc.offset, [rc.ap[0], rc.ap[1], [0, 4]])
            W_all = mpool.tile([128, 4, 4], f32, name="W_all")
            nc.vector.tensor_tensor(W_all[:], e_all[:], rcv, op=ALU.mult)

            # diag(W[r, j*128+p]) tiles for the PE weighted sum
            for r in range(4):
                for j in range(4):
                    nc.vector.tensor_scalar_mul(
                        dg16[:, (r * 4 + j) * 128 : (r * 4 + j + 1) * 128],
                        ident16,
                        W_all[:, j, r : r + 1],
                    )

        # deferred x16 casts for the final columns (needed only by the last
        # phase-2 chunks), emitted here so they queue behind the MLP ops
        for par, cast_src, cast_dst in late_casts:
            if par == 0:
                nc.vector.tensor_copy(cast_dst, cast_src)
            else:
                nc.scalar.copy(cast_dst, cast_src)

        # ---------------- phase 2: gather fp16, PE-weighted sum, store ------
        with tc.tile_pool(name="pp2", bufs=8, space="PSUM") as pp2:
            gmap = []          # (col0, width, gather-output tile)
            gi = 0
            gcol = 0
            par = 0
            for c0, w in PCHUNKS:
                while gi < len(GATHERS) and GATHERS[gi][0] <= c0:
                    g0, gw = GATHERS[gi]
                    gn = NRG * gw
                    gt = gpool.tile(
                        [128, 4, gn], f16, name=f"gt{gi}",
                        tag=f"g{gw}", bufs=GBUFS[gw],
                    )
                    nc.gpsimd.dma_gather(
                        gt[:],
                        xt[:],
                        idx_t[:, gcol : gcol + gn // 16],
                        gn,
                        gn,
                        BC,
                        transpose=True,
                        sbuf_tokens_per_rank=128,
                        sbuf_free_dim_per_rank=BC * 2,
                    )
                    gmap.append((g0, gw, gt))
                    gcol += gn // 16
                    gi += 1
                g0, gw, gt = next(g for g in reversed(gmap) if g[0] <= c0)
                off = c0 - g0
                ot = opool.tile([128, 4, NCH], f16, name="ot")
                # j-pair PSUM tiles (1 bank each, 8 bufs) so drains recycle
                # PSUM at half-chunk granularity and PE never backlogs
                for jp in range(2):
                    po = pp2.tile([128, 2, NCH], f32, name="po")
                    for jj in range(2):
                        j = 2 * jp + jj
                        for r in range(4):
                            if skip_r0 and r == 0:
                                rhs = x16[:, j, c0 : c0 + w]
                            else:
                                ri = r - 1 if skip_r0 else r
                                rhs = gt[:, j, ri * gw + off : ri * gw + off + w]
                            nc.tensor.matmul(
                                po[:, jj, :w],
                                dg16[:, (r * 4 + j) * 128 : (r * 4 + j + 1) * 128],
                                rhs,
                                start=(r == 0),
                                stop=(r == 3),
                                skip_group_check=True,
                            )
                    dst = ot[:, 2 * jp : 2 * jp + 2, :w]
                    if par % 2 == 0:
                        nc.scalar.copy(dst, po[:, :, :w])
                    else:
                        nc.vector.tensor_copy(dst, po[:, :, :w])
                    par += 1
                nc.sync.dma_start(out3[:, :, c0 : c0 + w], ot[:, :, :w])

    nc.compile()
    return nc


def _host_prep(x, rot_idx, w1, b1, w2, b2):
    x = np.asarray(x, dtype=np.float32)
    rot_idx = np.asarray(rot_idx, dtype=np.int64)
    w1 = np.asarray(w1, dtype=np.float32)
    b1 = np.asarray(b1, dtype=np.float32)
    w2 = np.asarray(w2, dtype=np.float32)
    b2 = np.asarray(b2, dtype=np.float32)

    skip_r0 = bool(np.array_equal(rot_idx[0], np.arange(L)))
    NRG = 3 if skip_r0 else 4

    cnt = np.zeros((R, L), dtype=np.float32)
    for r in range(R):
        cnt[r] = np.bincount(rot_idx[r], minlength=L).astype(np.float32)
    cnt /= np.float32(L)
    # cnt16[p, 4t+r] = cnt[r, t*128+p] (counts/L are exact in fp16)
    cnt_sb = np.ascontiguousarray(
        cnt.T.reshape(NT, 128, R).transpose(1, 0, 2).reshape(128, 128)
    )

    cst = np.zeros((128, CST_W), dtype=np.float32)
    cst[:, CST_ID : CST_ID + 128] = np.eye(128, dtype=np.float32)
    cst[:, CST_W1 : CST_W1 + 32] = (
        w1.reshape(2, 128, RED).transpose(1, 0, 2).reshape(128, 2 * RED)
    )
    cst[:, CST_B2 : CST_B2 + 2] = b2.reshape(2, 128).T
    cst[0:16, CST_W2 : CST_W2 + 256] = w2
    cst[0:16, CST_B1] = b1

    cst16 = np.zeros((128, C16_W), dtype=np.float16)
    cst16[:, C16_ID : C16_ID + 128] = np.eye(128, dtype=np.float16)
    cst16[:, C16_CNT : C16_CNT + 128] = cnt_sb.astype(np.float16)

    # gather index table: per gather call (col0, gw), linear order
    # [r1 l's, r2 l's, r3 l's] (plus r0 first when not skipped), wrapped
    # idx[p, s] = lin[s*16 + p], replicated over 8 groups of 16 partitions
    idx_sb = np.zeros((128, NRG * L // 16), dtype=np.int16)
    rlist = range(1, R) if skip_r0 else range(R)
    gcol = 0
    for g0, gw in GATHERS:
        gc = NRG * gw // 16
        lin = np.concatenate(
            [rot_idx[r, g0 : g0 + gw] for r in rlist]
        ).astype(np.int16)
        block = lin.reshape(gc, 16).T  # [16, gc]
        idx_sb[:, gcol : gcol + gc] = np.tile(block, (8, 1))
        gcol += gc

    shared = {"cst": cst, "cst16": cst16, "idx": idx_sb}
    in_maps = []
    for c in range(NCORES):
        mm = dict(shared)
        mm["x"] = np.ascontiguousarray(x[c * BPC : (c + 1) * BPC].reshape(BC, L))
        in_maps.append(mm)
    return skip_r0, in_maps


def kernel(x, rot_idx, w1, b1, w2, b2, _trace=False):
    from concourse import bass_utils

    skip_r0, in_maps = _host_prep(x, rot_idx, w1, b1, w2, b2)
    key = ("nc", skip_r0)
    if key not in _NC_CACHE:
        _NC_CACHE[key] = _build_nc(skip_r0)
    nc = _NC_CACHE[key]
    _NC_CACHE["nc"] = nc  # for test harness TimelineSim access
    res = bass_utils.run_bass_kernel_spmd(
        nc, in_maps, core_ids=list(range(NCORES)), trace=_trace
    )
    out = np.empty((B, C, L), dtype=np.float32)
    for c in range(NCORES):
        # device stores fp16 (within tolerance); widen during the unshard
        out[c * BPC : (c + 1) * BPC] = (
            res.results[c]["out"].astype(np.float32).reshape(BPC, C, L)
        )
    if _trace:
        kernel.last_results = res
    return out


# revision 31
# speedup vs baseline: 1.0120x; 1.0120x over previous
"""Trainium2 Bass kernel for CyclicShiftConv (Hilbert-rotation SE attention).

out[b,c,l] = sum_r softmax_r(MLP(mean_l x[b,c,rot_idx[r,l]]))[b,c,r] * x[b,c,rot_idx[r,l]]

Strategy (8 cores, data-parallel over batch; 2 samples = 512 (b,c) rows/core).
The kernel is DMA-bound (360 GB/s shared DMA fabric), so the design minimizes
bytes moved:
  - x (8 MiB f32/core) is loaded once, PE-transposed into an fp16 x^T resident
    in SBUF (no DRAM round-trip for the transposed copy).
  - mean over l of the gathered tensor == x^T contracted with bincount/L, so
    the SE-MLP input comes from tiny PE matmuls against an fp16 count vector.
  - The rotation gathers run SBUF->SBUF with the fp16 transpose-mode
    dma_gather (tokens_per_rank=128 maps token l' to partition l'%128 / rank
    l'//128, exactly the x^T stripe layout).  Transpose-mode writes return the
    data in natural [bc, l] layout, so no transpose-back matmuls are needed.
    fp16 halves gather bytes vs f32.
  - rot_idx[0] is the identity for the Hilbert rotation set; when the host
    detects this, rotation 0 is not gathered at all - its contribution is
    taken from the natural-layout f32 x directly (exact, and 4 MiB less DMA).
  - The softmax-weighted sum over rotations runs on the PE as matmuls against
    diagonal weight matrices accumulating in PSUM (out_j = sum_r
    diag(W_r) @ g_r), keeping the elementwise engines free.
"""

import sys

for _p in ("/opt/trn_rl_repo", "/opt/pypackages"):
    if _p not in sys.path:
        sys.path.append(_p)

import numpy as np

B, C, L, R, RED = 16, 256, 4096, 4, 16
NCORES = 8
BPC = B // NCORES          # samples per core
BC = BPC * C               # 512 rows per core
NT = L // 128              # 32 transpose stripes
NCH = 256                  # phase-2 max l-positions per processing chunk
# gather calls (col0, width): big early for DMA density / low desc-gen
# overhead, small late so the post-last-gather tail is short
GATHERS = [(i * 256, 256) for i in range(15)] + [(3840, 128), (3968, 128)]
# processing chunks (col0, width): PE/drain/store granularity
PCHUNKS = [(i * 256, 256) for i in range(15)] + [(3840, 128), (3968, 128)]
GBUFS = {256: 6, 128: 2}   # gather-buffer ring depth per width

# f32 constant block: [128, CST_W]
CST_ID = 0                 # identity [128, 128]
CST_W1 = 128               # w1       [128, 32]
CST_B2 = 160               # b2       [128, 2]
CST_W2 = 162               # w2       [16, 256] (partitions 0:16)
CST_B1 = 418               # b1       [16, 1]
CST_W = 419

# fp16 constant block: [128, 256]
C16_ID = 0                 # identity [128, 128]
C16_CNT = 128              # cnt      [128, 128]: cnt16[p, 4t+r] = cnt[r, t*128+p]
C16_W = 256

_NC_CACHE = {}


def _build_nc(skip_r0, loop_n=1):
    import concourse.bass as bass
    import concourse.mybir as mybir
    from concourse import bacc
    from concourse.tile import TileContext
    from contextlib import ExitStack

    f32 = mybir.dt.float32
    f16 = mybir.dt.float16
    i16 = mybir.dt.int16
    AF = mybir.ActivationFunctionType
    AX = mybir.AxisListType
    ALU = mybir.AluOpType

    NRG = 3 if skip_r0 else 4      # rotations gathered
    NIDXC = NRG * L // 16          # idx table columns

    f32r = mybir.dt.float32r

    nc = bacc.Bacc(
        "TRN2",
        target_bir_lowering=False,
        debug=False,
        enable_asserts=False,
        num_devices=NCORES,
    )

    x_in = nc.dram_tensor("x", [BC, L], f32, kind="ExternalInput").ap()
    cst_in = nc.dram_tensor("cst", [128, CST_W], f32, kind="ExternalInput").ap()
    cst16_in = nc.dram_tensor("cst16", [128, C16_W], f16, kind="ExternalInput").ap()
    idx_in = nc.dram_tensor("idx", [128, NIDXC], i16, kind="ExternalInput").ap()
    out = nc.dram_tensor("out", [BC, L], f16, kind="ExternalOutput").ap()

    x3 = x_in.rearrange("(j p) l -> p j l", j=4)
    out3 = out.rearrange("(j p) l -> p j l", j=4)

    with TileContext(nc) as tc, ExitStack() as ctx:
        if loop_n > 1:
            ctx.enter_context(tc.For_i(0, loop_n, 1))
        cpool = ctx.enter_context(tc.tile_pool(name="consts", bufs=1))
        xpool = ctx.enter_context(tc.tile_pool(name="xp", bufs=1))
        # phase-2 pools opened up-front so their SBUF ranges never overlap
        # the phase-1/MLP pools (address reuse creates false WAR waits)
        gpool = ctx.enter_context(tc.tile_pool(name="gp", bufs=6))
        opool = ctx.enter_context(tc.tile_pool(name="op", bufs=3))

        cst = cpool.tile([128, CST_W], f32, name="cst")
        nc.sync.dma_start(cst[:], cst_in)
        cst16 = cpool.tile([128, C16_W], f16, name="cst16")
        idx_t = cpool.tile([128, NIDXC], i16, name="idx_t")

        ident = cst[:, CST_ID : CST_ID + 128]
        w1_t = cst[:, CST_W1 : CST_W1 + 32]
        b2_t = cst[:, CST_B2 : CST_B2 + 2]
        w2_t = cst[0:16, CST_W2 : CST_W2 + 256]
        b1_t = cst[0:16, CST_B1 : CST_B1 + 1]
        ident16 = cst16[:, C16_ID : C16_ID + 128]
        cnt16 = cst16[:, C16_CNT : C16_CNT + 128]

        # x natural fp16: [128, j, L], bc = j*128 + p (kept through phase 2)
        x16 = xpool.tile([128, 4, L], f16, name="x16")
        # x^T fp16: xt[p, t*512 + k] = x[bc=k, l=t*128+p]
        xt = xpool.tile([128, NT * BC], f16, name="xt")
        s_sb = cpool.tile([4, BC], f32, name="s_sb")
        # diagonal softmax-weight matrices for the PE weighted sum
        dg16 = cpool.tile([128, 4 * 4 * 128], f16, name="dg16")

        # ---------------- phase 1: load, transpose -> xt fp16, s = x @ cnt --
        # late-half x16 casts deferred past the MLP: (engine-parity, src AP,
        # dst AP) emitted after the softmax so the cast queue never delays it
        late_casts = []
        with (
            tc.tile_pool(name="xsp", bufs=6) as xspool,
            tc.tile_pool(name="pp1", bufs=4, space="PSUM") as pp1,
            tc.tile_pool(name="pps", bufs=1, space="PSUM") as pps,
        ):
            # shrinking load chunks: the final small chunks minimize the
            # last-stripe transpose+drain latency that gates the first gather
            widths = [512] * 7 + [256, 256]
            col = 0
            tglob = 0
            for h, w in enumerate(widths):
                if h >= 7:
                    # final chunks live in the persistent pool: their x16
                    # casts are deferred past the MLP (see late_casts)
                    xh = xpool.tile([128, 4, w], f32, name=f"xh_l{h}")
                else:
                    xh = xspool.tile([128, 4, w], f32, name="xh")
                nc.sync.dma_start(xh[:], x3[:, :, col : col + w])
                if h == 0:
                    # constants not needed until mid-phase; keep x loads first
                    nc.sync.dma_start(cst16[:], cst16_in)
                    nc.sync.dma_start(idx_t[:], idx_in)
                for tt in range(w // 128):
                    t = tglob + tt
                    pt = pp1.tile([128, BC], f32, name="pt")
                    for j in range(4):
                        # fp32r transpose (1.5 cyc/row vs 2): >=16 mantissa
                        # bits survive, and xt is rounded to fp16 anyway
                        nc.tensor.transpose(
                            pt[:, j * 128 : (j + 1) * 128].bitcast(f32r),
                            xh[:, j, tt * 128 : (tt + 1) * 128].bitcast(f32r),
                            ident.bitcast(f32r),
                        )
                    xt_t = xt[:, t * BC : (t + 1) * BC]
                    if t % 2 == 0:
                        nc.vector.tensor_copy(xt_t, pt[:])
                    else:
                        nc.scalar.copy(xt_t, pt[:])
                # natural fp16 copy (consumed by phase 2's rotation-0 term);
                # NCH-aligned pieces, emitted after the drains so they never
                # head-of-line-block an xt drain in the engine queues
                for piece in range(w // NCH):
                    cast_src = xh[:, :, piece * NCH : (piece + 1) * NCH]
                    cast_dst = x16[:, :, col + piece * NCH : col + (piece + 1) * NCH]
                    if h < 7:
                        if (h + piece) % 2 == 0:
                            nc.vector.tensor_copy(cast_dst, cast_src)
                        else:
                            nc.scalar.copy(cast_dst, cast_src)
                    else:
                        late_casts.append(((h + piece) % 2, cast_src, cast_dst))
                col += w
                tglob += w // 128
            # batched mean matmuls (decoupled from the per-stripe chain so the
            # PE never stalls on a drain mid-phase)
            psum_s = pps.tile([4, BC], f32, name="psum_s")
            for t in range(NT):
                nc.tensor.matmul(
                    psum_s[:],
                    cnt16[:, 4 * t : 4 * t + 4],
                    xt[:, t * BC : (t + 1) * BC],
                    start=(t == 0),
                    stop=(t == NT - 1),
                )
            nc.vector.tensor_copy(s_sb[:], psum_s[:])

        # ---------------- SE MLP + softmax over rotations -------------------
        with (
            tc.tile_pool(name="mlp", bufs=1) as mpool,
            tc.tile_pool(name="ppm", bufs=1, space="PSUM") as ppm,
        ):
            sT = []
            for j in range(4):
                p_sT = ppm.tile([128, 4], f32, name="p_sT")
                nc.tensor.transpose(
                    p_sT[:], s_sb[:, j * 128 : (j + 1) * 128], cst[0:4, CST_ID : CST_ID + 4]
                )
                sTj = mpool.tile([128, 4], f32, name=f"sT{j}")
                nc.vector.tensor_copy(sTj[:], p_sT[:])
                sT.append(sTj)
            hs = []
            for b in range(BPC):
                p_h = ppm.tile([16, 4], f32, name="p_h")
                for hi in range(2):
                    nc.tensor.matmul(
                        p_h[:],
                        w1_t[:, hi * 16 : (hi + 1) * 16],
                        sT[2 * b + hi][:],
                        start=(hi == 0),
                        stop=(hi == 1),
                    )
                h_sb = mpool.tile([16, 4], f32, name=f"h{b}")
                nc.scalar.activation(h_sb[:], p_h[:], AF.Relu, bias=b1_t)
                hs.append(h_sb)
            p_sc = ppm.tile([128, 16], f32, name="p_sc")
            for b in range(BPC):
                for hi in range(2):
                    j = 2 * b + hi
                    nc.tensor.matmul(
                        p_sc[:, 4 * j : 4 * j + 4],
                        w2_t[:, hi * 128 : (hi + 1) * 128],
                        hs[b][:],
                        start=True, stop=True,
                    )
            sc_all = mpool.tile([128, 4, 4], f32, name="sc_all")
            b2ap = b2_t
            b2v = bass.AP(
                b2ap.tensor, b2ap.offset, [b2ap.ap[0], [0, 2], b2ap.ap[1], [0, 4]]
            )
            nc.vector.tensor_tensor(
                sc_all[:].rearrange("p (b hi) r -> p b hi r", b=2),
                p_sc[:].rearrange("p (b hi r) -> p b hi r", b=2, hi=2),
                b2v,
                op=ALU.add,
            )
            # no max-subtraction: scores are mean-pooled SE-MLP outputs with
            # |score| << 80, so exp cannot overflow in f32
            e_all = mpool.tile([128, 4, 4], f32, name="e_all")
            nc.scalar.activation(
                e_all[:].rearrange("p a r -> p (a r)"),
                sc_all[:].rearrange("p a r -> p (a r)"),
                AF.Exp,
            )
            sm = mpool.tile([128, 4], f32, name="sm")
            nc.vector.reduce_sum(sm[:], e_all[:], axis=AX.X)
            rcp = mpool.tile([128, 4], f32, name="rcp")
            nc.vector.reciprocal(rcp[:], sm[:])
            rc = rcp[:]
            rcv = bass.AP(rc.tensor, rc.offset, [rc.ap[0], rc.ap[1], [0, 4]])
            W_all = mpool.tile([128, 4, 4], f32, name="W_all")
            nc.vector.tensor_tensor(W_all[:], e_all[:], rcv, op=ALU.mult)

            # diag(W[r, j*128+p]) tiles for the PE weighted sum
            for r in range(4):
                for j in range(4):
                    nc.vector.tensor_scalar_mul(
                        dg16[:, (r * 4 + j) * 128 : (r * 4 + j + 1) * 128],
                        ident16,
                        W_all[:, j, r : r + 1],
                    )

        # deferred x16 casts for the final columns (needed only by the last
        # phase-2 chunks), emitted here so they queue behind the MLP ops
        for par, cast_src, cast_dst in late_casts:
            if par == 0:
                nc.vector.tensor_copy(cast_dst, cast_src)
            else:
                nc.scalar.copy(cast_dst, cast_src)

        # ---------------- phase 2: gather fp16, PE-weighted sum, store ------
        with tc.tile_pool(name="pp2", bufs=8, space="PSUM") as pp2:
            gmap = []          # (col0, width, gather-output tile)
            gi = 0
            gcol = 0
            par = 0
            for c0, w in PCHUNKS:
                while gi < len(GATHERS) and GATHERS[gi][0] <= c0:
                    g0, gw = GATHERS[gi]
                    gn = NRG * gw
                    gt = gpool.tile(
                        [128, 4, gn], f16, name=f"gt{gi}",
                        tag=f"g{gw}", bufs=GBUFS[gw],
                    )
                    nc.gpsimd.dma_gather(
                        gt[:],
                        xt[:],
                        idx_t[:, gcol : gcol + gn // 16],
                        gn,
                        gn,
                        BC,
                        transpose=True,
                        sbuf_tokens_per_rank=128,
                        sbuf_free_dim_per_rank=BC * 2,
                    )
                    gmap.append((g0, gw, gt))
                    gcol += gn // 16
                    gi += 1
                g0, gw, gt = next(g for g in reversed(gmap) if g[0] <= c0)
                off = c0 - g0
                ot = opool.tile([128, 4, NCH], f16, name="ot")
                # j-pair PSUM tiles (1 bank each, 8 bufs) so drains recycle
                # PSUM at half-chunk granularity and PE never backlogs
                for jp in range(2):
                    po = pp2.tile([128, 2, NCH], f32, name="po")
                    for jj in range(2):
                        j = 2 * jp + jj
                        for r in range(4):
                            if skip_r0 and r == 0:
                                rhs = x16[:, j, c0 : c0 + w]
                            else:
                                ri = r - 1 if skip_r0 else r
                                rhs = gt[:, j, ri * gw + off : ri * gw + off + w]
                            nc.tensor.matmul(
                                po[:, jj, :w],
                                dg16[:, (r * 4 + j) * 128 : (r * 4 + j + 1) * 128],
                                rhs,
                                start=(r == 0),
                                stop=(r == 3),
                                skip_group_check=True,
                            )
                    dst = ot[:, 2 * jp : 2 * jp + 2, :w]
                    if par % 2 == 0:
                        nc.scalar.copy(dst, po[:, :, :w])
                    else:
                        nc.vector.tensor_copy(dst, po[:, :, :w])
                    par += 1
                nc.sync.dma_start(out3[:, :, c0 : c0 + w], ot[:, :, :w])

    nc.compile()
    return nc


def _host_prep(x, rot_idx, w1, b1, w2, b2):
    x = np.asarray(x, dtype=np.float32)
    rot_idx = np.asarray(rot_idx, dtype=np.int64)
    w1 = np.asarray(w1, dtype=np.float32)
    b1 = np.asarray(b1, dtype=np.float32)
    w2 = np.asarray(w2, dtype=np.float32)
    b2 = np.asarray(b2, dtype=np.float32)

    skip_r0 = bool(np.array_equal(rot_idx[0], np.arange(L)))
    NRG = 3 if skip_r0 else 4

    cnt = np.zeros((R, L), dtype=np.float32)
    for r in range(R):
        cnt[r] = np.bincount(rot_idx[r], minlength=L).astype(np.float32)
    cnt /= np.float32(L)
    # cnt16[p, 4t+r] = cnt[r, t*128+p] (counts/L are exact in fp16)
    cnt_sb = np.ascontiguousarray(
        cnt.T.reshape(NT, 128, R).transpose(1, 0, 2).reshape(128, 128)
    )

    cst = np.zeros((128, CST_W), dtype=np.float32)
    cst[:, CST_ID : CST_ID + 128] = np.eye(128, dtype=np.float32)
    cst[:, CST_W1 : CST_W1 + 32] = (
        w1.reshape(2, 128, RED).transpose(1, 0, 2).reshape(128, 2 * RED)
    )
    cst[:, CST_B2 : CST_B2 + 2] = b2.reshape(2, 128).T
    cst[0:16, CST_W2 : CST_W2 + 256] = w2
    cst[0:16, CST_B1] = b1

    cst16 = np.zeros((128, C16_W), dtype=np.float16)
    cst16[:, C16_ID : C16_ID + 128] = np.eye(128, dtype=np.float16)
    cst16[:, C16_CNT : C16_CNT + 128] = cnt_sb.astype(np.float16)

    # gather index table: per gather call (col0, gw), linear order
    # [r1 l's, r2 l's, r3 l's] (plus r0 first when not skipped), wrapped
    # idx[p, s] = lin[s*16 + p], replicated over 8 groups of 16 partitions
    idx_sb = np.zeros((128, NRG * L // 16), dtype=np.int16)
    rlist = range(1, R) if skip_r0 else range(R)
    gcol = 0
    for g0, gw in GATHERS:
        gc = NRG * gw // 16
        lin = np.concatenate(
            [rot_idx[r, g0 : g0 + gw] for r in rlist]
        ).astype(np.int16)
        block = lin.reshape(gc, 16).T  # [16, gc]
        idx_sb[:, gcol : gcol + gc] = np.tile(block, (8, 1))
        gcol += gc

    shared = {"cst": cst, "cst16": cst16, "idx": idx_sb}
    in_maps = []
    for c in range(NCORES):
        mm = dict(shared)
        mm["x"] = np.ascontiguousarray(x[c * BPC : (c + 1) * BPC].reshape(BC, L))
        in_maps.append(mm)
    return skip_r0, in_maps


def kernel(x, rot_idx, w1, b1, w2, b2, _trace=False):
    from concourse import bass_utils

    skip_r0, in_maps = _host_prep(x, rot_idx, w1, b1, w2, b2)
    key = ("nc", skip_r0)
    if key not in _NC_CACHE:
        _NC_CACHE[key] = _build_nc(skip_r0)
    nc = _NC_CACHE[key]
    _NC_CACHE["nc"] = nc  # for test harness TimelineSim access
    res = bass_utils.run_bass_kernel_spmd(
        nc, in_maps, core_ids=list(range(NCORES)), trace=_trace
    )
    out = np.empty((B, C, L), dtype=np.float32)
    for c in range(NCORES):
        # device stores fp16 (within tolerance); widen during the unshard
        out[c * BPC : (c + 1) * BPC] = (
            res.results[c]["out"].astype(np.float32).reshape(BPC, C, L)
        )
    if _trace:
        kernel.last_results = res
    return out


# revision 39
# speedup vs baseline: 1.1027x; 1.0895x over previous
"""Trainium2 Bass kernel for CyclicShiftConv (Hilbert-rotation SE attention).

out[b,c,l] = sum_r softmax_r(MLP(mean_l x[b,c,rot_idx[r,l]]))[b,c,r] * x[b,c,rot_idx[r,l]]

Strategy (8 cores, data-parallel over batch; 2 samples = 512 (b,c) rows/core).
The kernel is DMA-bound (360 GB/s shared DMA fabric), so the design minimizes
bytes moved:
  - x (8 MiB f32/core) is loaded once, PE-transposed into an fp16 x^T resident
    in SBUF (no DRAM round-trip for the transposed copy).
  - mean over l of the gathered tensor == x^T contracted with bincount/L, so
    the SE-MLP input comes from tiny PE matmuls against an fp16 count vector.
  - The rotation gathers run SBUF->SBUF with the fp16 transpose-mode
    dma_gather (tokens_per_rank=128 maps token l' to partition l'%128 / rank
    l'//128, exactly the x^T stripe layout).  Transpose-mode writes return the
    data in natural [bc, l] layout, so no transpose-back matmuls are needed.
    fp16 halves gather bytes vs f32.
  - rot_idx[0] is the identity for the Hilbert rotation set; when the host
    detects this, rotation 0 is not gathered at all - its contribution is
    taken from the natural-layout f32 x directly (exact, and 4 MiB less DMA).
  - The softmax-weighted sum over rotations runs on the PE as matmuls against
    diagonal weight matrices accumulating in PSUM (out_j = sum_r
    diag(W_r) @ g_r), keeping the elementwise engines free.
"""

import sys

for _p in ("/opt/trn_rl_repo", "/opt/pypackages"):
    if _p not in sys.path:
        sys.path.append(_p)

import numpy as np

B, C, L, R, RED = 16, 256, 4096, 4, 16
NCORES = 8
BPC = B // NCORES          # samples per core
BC = BPC * C               # 512 rows per core
NT = L // 128              # 32 transpose stripes
NCH = 256                  # phase-2 max l-positions per processing chunk
# gather calls (col0, width): big early for DMA density / low desc-gen
# overhead, small late so the post-last-gather tail is short
GATHERS = [(i * 256, 256) for i in range(16)]
# processing chunks (col0, width): PE/drain/store granularity
PCHUNKS = [(i * 256, 256) for i in range(16)]
GBUFS = {256: 6}   # gather-buffer ring depth per width

# f32 constant block: [128, CST_W]
CST_ID = 0                 # identity [128, 128]
CST_W1 = 128               # w1       [128, 32]
CST_B2 = 160               # b2       [128, 2]
CST_W2 = 162               # w2       [16, 256] (partitions 0:16)
CST_B1 = 418               # b1       [16, 1]
CST_W = 419

# fp16 constant block: [128, 256]
C16_ID = 0                 # identity [128, 128]
C16_CNT = 128              # cnt      [128, 128]: cnt16[p, 4t+r] = cnt[r, t*128+p]
C16_W = 256

_NC_CACHE = {}


def _build_nc(skip_r0, loop_n=1):
    import concourse.bass as bass
    import concourse.mybir as mybir
    from concourse import bacc
    from concourse.tile import TileContext
    from contextlib import ExitStack

    f32 = mybir.dt.float32
    f16 = mybir.dt.float16
    i16 = mybir.dt.int16
    AF = mybir.ActivationFunctionType
    AX = mybir.AxisListType
    ALU = mybir.AluOpType

    NRG = 3 if skip_r0 else 4      # rotations gathered
    NIDXC = NRG * L // 16          # idx table columns

    f32r = mybir.dt.float32r

    nc = bacc.Bacc(
        "TRN2",
        target_bir_lowering=False,
        debug=False,
        enable_asserts=False,
        num_devices=NCORES,
    )

    x_in = nc.dram_tensor("x", [BC, L], f32, kind="ExternalInput").ap()
    cst_in = nc.dram_tensor("cst", [128, CST_W], f32, kind="ExternalInput").ap()
    cst16_in = nc.dram_tensor("cst16", [128, C16_W], f16, kind="ExternalInput").ap()
    idx_in = nc.dram_tensor("idx", [128, NIDXC], i16, kind="ExternalInput").ap()
    out = nc.dram_tensor("out", [BC, L], f16, kind="ExternalOutput").ap()

    x3 = x_in.rearrange("(j p) l -> p j l", j=4)
    out3 = out.rearrange("(j p) l -> p j l", j=4)

    with TileContext(nc) as tc, ExitStack() as ctx:
        if loop_n > 1:
            ctx.enter_context(tc.For_i(0, loop_n, 1))
        cpool = ctx.enter_context(tc.tile_pool(name="consts", bufs=1))
        xpool = ctx.enter_context(tc.tile_pool(name="xp", bufs=1))
        # phase-2 pools opened up-front so their SBUF ranges never overlap
        # the phase-1/MLP pools (address reuse creates false WAR waits)
        gpool = ctx.enter_context(tc.tile_pool(name="gp", bufs=6))
        opool = ctx.enter_context(tc.tile_pool(name="op", bufs=6))

        cst = cpool.tile([128, CST_W], f32, name="cst")
        nc.sync.dma_start(cst[:], cst_in)
        cst16 = cpool.tile([128, C16_W], f16, name="cst16")
        idx_t = cpool.tile([128, NIDXC], i16, name="idx_t")

        ident = cst[:, CST_ID : CST_ID + 128]
        w1_t = cst[:, CST_W1 : CST_W1 + 32]
        b2_t = cst[:, CST_B2 : CST_B2 + 2]
        w2_t = cst[0:16, CST_W2 : CST_W2 + 256]
        b1_t = cst[0:16, CST_B1 : CST_B1 + 1]
        ident16 = cst16[:, C16_ID : C16_ID + 128]
        cnt16 = cst16[:, C16_CNT : C16_CNT + 128]

        # x natural fp16: [128, j, L], bc = j*128 + p (kept through phase 2)
        x16 = xpool.tile([128, 4, L], f16, name="x16")
        # x^T fp16: xt[p, t*512 + k] = x[bc=k, l=t*128+p]
        xt = xpool.tile([128, NT * BC], f16, name="xt")
        s_sb = cpool.tile([4, BC], f32, name="s_sb")
        # diagonal softmax-weight matrices for the PE weighted sum
        dg16 = cpool.tile([128, 4 * 4 * 128], f16, name="dg16")

        # ---------------- phase 1: load, transpose -> xt fp16, s = x @ cnt --
        # late-half x16 casts deferred past the MLP: (engine-parity, src AP,
        # dst AP) emitted after the softmax so the cast queue never delays it
        late_casts = []
        with (
            tc.tile_pool(name="xsp", bufs=6) as xspool,
            tc.tile_pool(name="pp1", bufs=4, space="PSUM") as pp1,
            tc.tile_pool(name="pps", bufs=1, space="PSUM") as pps,
        ):
            # shrinking load chunks: the final small chunks minimize the
            # last-stripe transpose+drain latency that gates the first gather
            widths = [512] * 7 + [256, 256]
            col = 0
            tglob = 0
            for h, w in enumerate(widths):
                if h >= 7:
                    # final chunks live in the persistent pool: their x16
                    # casts are deferred past the MLP (see late_casts)
                    xh = xpool.tile([128, 4, w], f32, name=f"xh_l{h}")
                else:
                    xh = xspool.tile([128, 4, w], f32, name="xh")
                if h == 0:
                    # first load issued from ACT's DGE so its descriptor prep
                    # overlaps SP's cst issue instead of queueing behind it
                    nc.scalar.dma_start(xh[:], x3[:, :, col : col + w])
                    # constants not needed until mid-phase; keep x loads first
                    nc.sync.dma_start(cst16[:], cst16_in)
                    nc.sync.dma_start(idx_t[:], idx_in)
                else:
                    nc.sync.dma_start(xh[:], x3[:, :, col : col + w])
                for tt in range(w // 128):
                    t = tglob + tt
                    pt = pp1.tile([128, BC], f32, name="pt")
                    for j in range(4):
                        nc.tensor.transpose(
                            pt[:, j * 128 : (j + 1) * 128],
                            xh[:, j, tt * 128 : (tt + 1) * 128],
                            ident,
                        )
                    xt_t = xt[:, t * BC : (t + 1) * BC]
                    if t % 2 == 0:
                        nc.vector.tensor_copy(xt_t, pt[:])
                    else:
                        nc.scalar.copy(xt_t, pt[:])
                # natural fp16 copy (consumed by phase 2's rotation-0 term);
                # NCH-aligned pieces, emitted after the drains so they never
                # head-of-line-block an xt drain in the engine queues
                for piece in range(w // NCH):
                    cast_src = xh[:, :, piece * NCH : (piece + 1) * NCH]
                    cast_dst = x16[:, :, col + piece * NCH : col + (piece + 1) * NCH]
                    if h < 7:
                        if (h + piece) % 2 == 0:
                            nc.vector.tensor_copy(cast_dst, cast_src)
                        else:
                            nc.scalar.copy(cast_dst, cast_src)
                    else:
                        late_casts.append(((h + piece) % 2, cast_src, cast_dst))
                col += w
                tglob += w // 128
            # batched mean matmuls (decoupled from the per-stripe chain so the
            # PE never stalls on a drain mid-phase)
            psum_s = pps.tile([4, BC], f32, name="psum_s")
            for t in range(NT):
                nc.tensor.matmul(
                    psum_s[:],
                    cnt16[:, 4 * t : 4 * t + 4],
                    xt[:, t * BC : (t + 1) * BC],
                    start=(t == 0),
                    stop=(t == NT - 1),
                )
            nc.vector.tensor_copy(s_sb[:], psum_s[:])

        # ---------------- SE MLP + softmax over rotations -------------------
        with (
            tc.tile_pool(name="mlp", bufs=1) as mpool,
            tc.tile_pool(name="ppm", bufs=1, space="PSUM") as ppm,
        ):
            sT = []
            for j in range(4):
                p_sT = ppm.tile([128, 4], f32, name="p_sT")
                nc.tensor.transpose(
                    p_sT[:], s_sb[:, j * 128 : (j + 1) * 128], cst[0:4, CST_ID : CST_ID + 4]
                )
                sTj = mpool.tile([128, 4], f32, name=f"sT{j}")
                nc.vector.tensor_copy(sTj[:], p_sT[:])
                sT.append(sTj)
            hs = []
            for b in range(BPC):
                p_h = ppm.tile([16, 4], f32, name="p_h")
                for hi in range(2):
                    nc.tensor.matmul(
                        p_h[:],
                        w1_t[:, hi * 16 : (hi + 1) * 16],
                        sT[2 * b + hi][:],
                        start=(hi == 0),
                        stop=(hi == 1),
                    )
                h_sb = mpool.tile([16, 4], f32, name=f"h{b}")
                nc.scalar.activation(h_sb[:], p_h[:], AF.Relu, bias=b1_t)
                hs.append(h_sb)
            p_sc = ppm.tile([128, 16], f32, name="p_sc")
            for b in range(BPC):
                for hi in range(2):
                    j = 2 * b + hi
                    nc.tensor.matmul(
                        p_sc[:, 4 * j : 4 * j + 4],
                        w2_t[:, hi * 128 : (hi + 1) * 128],
                        hs[b][:],
                        start=True, stop=True,
                    )
            sc_all = mpool.tile([128, 4, 4], f32, name="sc_all")
            b2ap = b2_t
            b2v = bass.AP(
                b2ap.tensor, b2ap.offset, [b2ap.ap[0], [0, 2], b2ap.ap[1], [0, 4]]
            )
            nc.vector.tensor_tensor(
                sc_all[:].rearrange("p (b hi) r -> p b hi r", b=2),
                p_sc[:].rearrange("p (b hi r) -> p b hi r", b=2, hi=2),
                b2v,
                op=ALU.add,
            )
            # no max-subtraction: scores are mean-pooled SE-MLP outputs with
            # |score| << 80, so exp cannot overflow in f32
            e_all = mpool.tile([128, 4, 4], f32, name="e_all")
            nc.scalar.activation(
                e_all[:].rearrange("p a r -> p (a r)"),
                sc_all[:].rearrange("p a r -> p (a r)"),
                AF.Exp,
            )
            sm = mpool.tile([128, 4], f32, name="sm")
            nc.vector.reduce_sum(sm[:], e_all[:], axis=AX.X)
            rcp = mpool.tile([128, 4], f32, name="rcp")
            nc.vector.reciprocal(rcp[:], sm[:])
            rc = rcp[:]
            rcv = bass.AP(rc.tensor, rc.offset, [rc.ap[0], rc.ap[1], [0, 4]])
            W_all = mpool.tile([128, 4, 4], f32, name="W_all")
            nc.vector.tensor_tensor(W_all[:], e_all[:], rcv, op=ALU.mult)

            # diag(W[r, j*128+p]) tiles for the PE weighted sum
            for r in range(4):
                for j in range(4):
                    nc.vector.tensor_scalar_mul(
                        dg16[:, (r * 4 + j) * 128 : (r * 4 + j + 1) * 128],
                        ident16,
                        W_all[:, j, r : r + 1],
                    )

        # deferred x16 casts for the final columns (needed only by the last
        # phase-2 chunks), emitted here so they queue behind the MLP ops
        for par, cast_src, cast_dst in late_casts:
            if par == 0:
                nc.vector.tensor_copy(cast_dst, cast_src)
            else:
                nc.scalar.copy(cast_dst, cast_src)

        # ---------------- phase 2: gather fp16, PE-weighted sum, store ------
        with tc.tile_pool(name="pp2", bufs=8, space="PSUM") as pp2:
            gmap = []          # (col0, width, gather-output tile)
            gi = 0
            gcol = 0
            par = 0
            for c0, w in PCHUNKS:
                while gi < len(GATHERS) and GATHERS[gi][0] <= c0:
                    g0, gw = GATHERS[gi]
                    gn = NRG * gw
                    gt = gpool.tile(
                        [128, 4, gn], f16, name=f"gt{gi}",
                        tag=f"g{gw}", bufs=GBUFS[gw],
                    )
                    nc.gpsimd.dma_gather(
                        gt[:],
                        xt[:],
                        idx_t[:, gcol : gcol + gn // 16],
                        gn,
                        gn,
                        BC,
                        transpose=True,
                        sbuf_tokens_per_rank=128,
                        sbuf_free_dim_per_rank=BC * 2,
                    )
                    gmap.append((g0, gw, gt))
                    gcol += gn // 16
                    gi += 1
                g0, gw, gt = next(g for g in reversed(gmap) if g[0] <= c0)
                off = c0 - g0
                ot = opool.tile([128, 4, NCH], f16, name="ot")
                # j-pair PSUM tiles (1 bank each, 8 bufs) so drains recycle
                # PSUM at half-chunk granularity and PE never backlogs
                for jp in range(2):
                    po = pp2.tile([128, 2, NCH], f32, name="po")
                    for jj in range(2):
                        j = 2 * jp + jj
                        for r in range(4):
                            if skip_r0 and r == 0:
                                rhs = x16[:, j, c0 : c0 + w]
                            else:
                                ri = r - 1 if skip_r0 else r
                                rhs = gt[:, j, ri * gw + off : ri * gw + off + w]
                            nc.tensor.matmul(
                                po[:, jj, :w],
                                dg16[:, (r * 4 + j) * 128 : (r * 4 + j + 1) * 128],
                                rhs,
                                start=(r == 0),
                                stop=(r == 3),
                                skip_group_check=True,
                            )
                    dst = ot[:, 2 * jp : 2 * jp + 2, :w]
                    if par % 2 == 0:
                        nc.scalar.copy(dst, po[:, :, :w])
                    else:
                        nc.vector.tensor_copy(dst, po[:, :, :w])
                    par += 1
                nc.sync.dma_start(out3[:, :, c0 : c0 + w], ot[:, :, :w])

    nc.compile()
    return nc


def _host_prep(x, rot_idx, w1, b1, w2, b2):
    x = np.asarray(x, dtype=np.float32)
    rot_idx = np.asarray(rot_idx, dtype=np.int64)
    w1 = np.asarray(w1, dtype=np.float32)
    b1 = np.asarray(b1, dtype=np.float32)
    w2 = np.asarray(w2, dtype=np.float32)
    b2 = np.asarray(b2, dtype=np.float32)

    skip_r0 = bool(np.array_equal(rot_idx[0], np.arange(L)))
    NRG = 3 if skip_r0 else 4

    cnt = np.zeros((R, L), dtype=np.float32)
    for r in range(R):
        cnt[r] = np.bincount(rot_idx[r], minlength=L).astype(np.float32)
    cnt /= np.float32(L)
    # cnt16[p, 4t+r] = cnt[r, t*128+p] (counts/L are exact in fp16)
    cnt_sb = np.ascontiguousarray(
        cnt.T.reshape(NT, 128, R).transpose(1, 0, 2).reshape(128, 128)
    )

    cst = np.zeros((128, CST_W), dtype=np.float32)
    cst[:, CST_ID : CST_ID + 128] = np.eye(128, dtype=np.float32)
    cst[:, CST_W1 : CST_W1 + 32] = (
        w1.reshape(2, 128, RED).transpose(1, 0, 2).reshape(128, 2 * RED)
    )
    cst[:, CST_B2 : CST_B2 + 2] = b2.reshape(2, 128).T
    cst[0:16, CST_W2 : CST_W2 + 256] = w2
    cst[0:16, CST_B1] = b1

    cst16 = np.zeros((128, C16_W), dtype=np.float16)
    cst16[:, C16_ID : C16_ID + 128] = np.eye(128, dtype=np.float16)
    cst16[:, C16_CNT : C16_CNT + 128] = cnt_sb.astype(np.float16)

    # gather index table: per gather call (col0, gw), linear order
    # [r1 l's, r2 l's, r3 l's] (plus r0 first when not skipped), wrapped
    # idx[p, s] = lin[s*16 + p], replicated over 8 groups of 16 partitions
    idx_sb = np.zeros((128, NRG * L // 16), dtype=np.int16)
    rlist = range(1, R) if skip_r0 else range(R)
    gcol = 0
    for g0, gw in GATHERS:
        gc = NRG * gw // 16
        lin = np.concatenate(
            [rot_idx[r, g0 : g0 + gw] for r in rlist]
        ).astype(np.int16)
        block = lin.reshape(gc, 16).T  # [16, gc]
        idx_sb[:, gcol : gcol + gc] = np.tile(block, (8, 1))
        gcol += gc

    shared = {"cst": cst, "cst16": cst16, "idx": idx_sb}
    in_maps = []
    for c in range(NCORES):
        mm = dict(shared)
        mm["x"] = np.ascontiguousarray(x[c * BPC : (c + 1) * BPC].reshape(BC, L))
        in_maps.append(mm)
    return skip_r0, in_maps


def kernel(x, rot_idx, w1, b1, w2, b2, _trace=False):
    from concourse import bass_utils

    skip_r0, in_maps = _host_prep(x, rot_idx, w1, b1, w2, b2)
    key = ("nc", skip_r0)
    if key not in _NC_CACHE:
        _NC_CACHE[key] = _build_nc(skip_r0)
    nc = _NC_CACHE[key]
    _NC_CACHE["nc"] = nc  # for test harness TimelineSim access
    res = bass_utils.run_bass_kernel_spmd(
        nc, in_maps, core_ids=list(range(NCORES)), trace=_trace
    )
    out = np.empty((B, C, L), dtype=np.float32)
    for c in range(NCORES):
        # device stores fp16 (within tolerance); widen during the unshard
        out[c * BPC : (c + 1) * BPC] = (
            res.results[c]["out"].astype(np.float32).reshape(BPC, C, L)
        )
    if _trace:
        kernel.last_results = res
    return out


# revision 52
# speedup vs baseline: 1.1214x; 1.0170x over previous
"""Trainium2 Bass kernel for CyclicShiftConv (Hilbert-rotation SE attention).

out[b,c,l] = sum_r softmax_r(MLP(mean_l x[b,c,rot_idx[r,l]]))[b,c,r] * x[b,c,rot_idx[r,l]]

Strategy (8 cores, data-parallel over batch; 2 samples = 512 (b,c) rows/core).
The kernel is DMA-bound (360 GB/s shared DMA fabric), so the design minimizes
bytes moved:
  - x (8 MiB f32/core) is loaded once, PE-transposed into an fp16 x^T resident
    in SBUF (no DRAM round-trip for the transposed copy).
  - mean over l of the gathered tensor == x^T contracted with bincount/L, so
    the SE-MLP input comes from tiny PE matmuls against an fp16 count vector.
  - The rotation gathers run SBUF->SBUF with the fp16 transpose-mode
    dma_gather (tokens_per_rank=128 maps token l' to partition l'%128 / rank
    l'//128, exactly the x^T stripe layout).  Transpose-mode writes return the
    data in natural [bc, l] layout, so no transpose-back matmuls are needed.
    fp16 halves gather bytes vs f32.
  - rot_idx[0] is the identity for the Hilbert rotation set; when the host
    detects this, rotation 0 is not gathered at all - its contribution comes
    from a natural-layout fp16 copy of x cast during phase 1 (4 MiB less DMA).
    Arbitrary (non-identity) index tables fall back to gathering all four
    rotations with 128-wide gather calls (512 descriptors each - a 1024-desc
    call would fill the SWDGE ring carveout and hang).
  - The softmax-weighted sum over rotations runs on the PE as matmuls against
    diagonal weight matrices accumulating in PSUM (out_j = sum_r
    diag(W_r) @ g_r), keeping the elementwise engines free for PSUM drains.
  - The output is stored fp16 on device (error ~3e-4 << the 2e-2 tolerance)
    and widened to f32 during the host-side unshard, halving output DMA.
"""

import sys

for _p in ("/opt/trn_rl_repo", "/opt/pypackages"):
    if _p not in sys.path:
        sys.path.append(_p)

import numpy as np

B, C, L, R, RED = 16, 256, 4096, 4, 16
NCORES = 8
BPC = B // NCORES          # samples per core
BC = BPC * C               # 512 rows per core
NT = L // 128              # 32 transpose stripes
NCH = 256                  # phase-2 max l-positions per processing chunk
# gather calls (col0, width): big early for DMA density / low desc-gen
# overhead, small late so the post-last-gather tail is short
GATHERS = [(i * 256, 256) for i in range(16)]
# processing chunks (col0, width): PE/drain/store granularity
PCHUNKS = [(i * 256, 256) for i in range(16)]
GBUFS = {256: 7}   # gather-buffer ring depth per width


def _plan(skip_r0):
    """Gather/processing chunking per variant. The 4-rotation fallback uses
    128-wide gathers: 4*256=1024 descriptors per call hits the SWDGE ring
    carveout (16384/16) and hangs the device; 4*128=512 is safe."""
    if skip_r0:
        return GATHERS, PCHUNKS, GBUFS
    g = [(i * 128, 128) for i in range(32)]
    return g, g, {128: 8}

# f32 constant block: [128, CST_W]
CST_ID = 0                 # identity [128, 128]
CST_W1 = 128               # w1       [128, 32]
CST_B2 = 160               # b2       [128, 2]
CST_W2 = 162               # w2       [16, 256] (partitions 0:16)
CST_B1 = 418               # b1       [16, 1]
CST_W = 419

# fp16 constant block: [128, 256]
C16_ID = 0                 # identity [128, 128]
C16_CNT = 128              # cnt      [128, 128]: cnt16[p, 4t+r] = cnt[r, t*128+p]
C16_W = 256

_NC_CACHE = {}


def _build_nc(skip_r0, loop_n=1):
    import concourse.bass as bass
    import concourse.mybir as mybir
    from concourse import bacc
    from concourse.tile import TileContext
    from contextlib import ExitStack

    f32 = mybir.dt.float32
    f16 = mybir.dt.float16
    i16 = mybir.dt.int16
    AF = mybir.ActivationFunctionType
    AX = mybir.AxisListType
    ALU = mybir.AluOpType

    NRG = 3 if skip_r0 else 4      # rotations gathered
    NIDXC = NRG * L // 16          # idx table columns

    f32r = mybir.dt.float32r

    nc = bacc.Bacc(
        "TRN2",
        target_bir_lowering=False,
        debug=False,
        enable_asserts=False,
        num_devices=NCORES,
    )

    x_in = nc.dram_tensor("x", [BC, L], f32, kind="ExternalInput").ap()
    cst_in = nc.dram_tensor("cst", [128, CST_W], f32, kind="ExternalInput").ap()
    cst16_in = nc.dram_tensor("cst16", [128, C16_W], f16, kind="ExternalInput").ap()
    idx_in = nc.dram_tensor("idx", [128, NIDXC], i16, kind="ExternalInput").ap()
    out = nc.dram_tensor("out", [BC, L], f16, kind="ExternalOutput").ap()

    x3 = x_in.rearrange("(j p) l -> p j l", j=4)
    out3 = out.rearrange("(j p) l -> p j l", j=4)

    with TileContext(nc) as tc, ExitStack() as ctx:
        if loop_n > 1:
            ctx.enter_context(tc.For_i(0, loop_n, 1))
        cpool = ctx.enter_context(tc.tile_pool(name="consts", bufs=1))
        xpool = ctx.enter_context(tc.tile_pool(name="xp", bufs=1))
        # phase-2 pools opened up-front so their SBUF ranges never overlap
        # the phase-1/MLP pools (address reuse creates false WAR waits)
        gpool = ctx.enter_context(tc.tile_pool(name="gp", bufs=6))
        opool = ctx.enter_context(tc.tile_pool(name="op", bufs=6))

        cst = cpool.tile([128, CST_W], f32, name="cst")
        nc.sync.dma_start(cst[:], cst_in)
        cst16 = cpool.tile([128, C16_W], f16, name="cst16")
        idx_t = cpool.tile([128, NIDXC], i16, name="idx_t")

        ident = cst[:, CST_ID : CST_ID + 128]
        w1_t = cst[:, CST_W1 : CST_W1 + 32]
        b2_t = cst[:, CST_B2 : CST_B2 + 2]
        w2_t = cst[0:16, CST_W2 : CST_W2 + 256]
        b1_t = cst[0:16, CST_B1 : CST_B1 + 1]
        ident16 = cst16[:, C16_ID : C16_ID + 128]
        cnt16 = cst16[:, C16_CNT : C16_CNT + 128]

        # x natural fp16: [128, j, L], bc = j*128 + p (kept through phase 2)
        x16 = xpool.tile([128, 4, L], f16, name="x16")
        # x^T fp16: xt[p, t*512 + k] = x[bc=k, l=t*128+p]
        xt = xpool.tile([128, NT * BC], f16, name="xt")
        s_sb = cpool.tile([4, BC], f32, name="s_sb")
        # diagonal softmax-weight matrices for the PE weighted sum
        dg16 = cpool.tile([128, 4 * 4 * 128], f16, name="dg16")

        # ---------------- phase 1: load, transpose -> xt fp16, s = x @ cnt --
        # late-half x16 casts deferred past the MLP: (engine-parity, src AP,
        # dst AP) emitted after the softmax so the cast queue never delays it
        late_casts = []
        with (
            tc.tile_pool(name="xsp", bufs=6) as xspool,
            tc.tile_pool(name="pp1", bufs=4, space="PSUM") as pp1,
            tc.tile_pool(name="pps", bufs=1, space="PSUM") as pps,
        ):
            # shrinking load chunks: the final small chunks minimize the
            # last-stripe transpose+drain latency that gates the first gather
            widths = [512] * 6 + [256] * 3 + [128, 128]
            col = 0
            tglob = 0
            for h, w in enumerate(widths):
                if h >= 9:
                    # final chunks live in the persistent pool: their x16
                    # casts are deferred past the MLP (see late_casts)
                    xh = xpool.tile([128, 4, w], f32, name=f"xh_l{h}")
                else:
                    xh = xspool.tile([128, 4, w], f32, name="xh")
                if h == 0:
                    # first load issued from ACT's DGE so its descriptor prep
                    # overlaps SP's cst issue instead of queueing behind it
                    nc.scalar.dma_start(xh[:], x3[:, :, col : col + w])
                    # cnt16 feeds the interleaved s-matmuls: keep it early
                    nc.sync.dma_start(cst16[:], cst16_in)
                else:
                    nc.sync.dma_start(xh[:], x3[:, :, col : col + w])
                if h == len(widths) - 1:
                    # idx is only read by gather desc-gen (~30us); loading it
                    # last shaves its slot off the x-load critical path
                    nc.sync.dma_start(idx_t[:], idx_in)
                for tt in range(w // 128):
                    t = tglob + tt
                    pt = pp1.tile([128, BC], f32, name="pt")
                    for j in range(4):
                        nc.tensor.transpose(
                            pt[:, j * 128 : (j + 1) * 128],
                            xh[:, j, tt * 128 : (tt + 1) * 128],
                            ident,
                        )
                    xt_t = xt[:, t * BC : (t + 1) * BC]
                    if t % 2 == 0:
                        nc.vector.tensor_copy(xt_t, pt[:])
                    else:
                        nc.scalar.copy(xt_t, pt[:])
                # natural fp16 copy (consumed by phase 2's rotation-0 term);
                # NCH-aligned pieces, emitted after the drains so they never
                # head-of-line-block an xt drain in the engine queues
                pw = min(w, NCH)
                for piece in range(w // pw):
                    cast_src = xh[:, :, piece * pw : (piece + 1) * pw]
                    cast_dst = x16[:, :, col + piece * pw : col + (piece + 1) * pw]
                    if h < 7:
                        if (h + piece) % 2 == 0:
                            nc.vector.tensor_copy(cast_dst, cast_src)
                        else:
                            nc.scalar.copy(cast_dst, cast_src)
                    else:
                        late_casts.append(((h + piece) % 2, cast_src, cast_dst))
                col += w
                tglob += w // 128
            # batched mean matmuls (decoupled from the per-stripe chain so the
            # PE never stalls on a drain mid-phase)
            psum_s = pps.tile([4, BC], f32, name="psum_s")
            for t in range(NT):
                nc.tensor.matmul(
                    psum_s[:],
                    cnt16[:, 4 * t : 4 * t + 4],
                    xt[:, t * BC : (t + 1) * BC],
                    start=(t == 0),
                    stop=(t == NT - 1),
                )
            nc.vector.tensor_copy(s_sb[:], psum_s[:])

        # ---------------- SE MLP + softmax over rotations -------------------
        with (
            tc.tile_pool(name="mlp", bufs=1) as mpool,
            tc.tile_pool(name="ppm", bufs=1, space="PSUM") as ppm,
        ):
            # one PSUM tile + single drain for all 4 s^T transposes
            p_sT4 = ppm.tile([128, 16], f32, name="p_sT4")
            for j in range(4):
                nc.tensor.transpose(
                    p_sT4[:, 4 * j : 4 * j + 4],
                    s_sb[:, j * 128 : (j + 1) * 128],
                    cst[0:4, CST_ID : CST_ID + 4],
                )
            sT_all = mpool.tile([128, 16], f32, name="sT_all")
            nc.vector.tensor_copy(sT_all[:], p_sT4[:])
            hs = []
            for b in range(BPC):
                p_h = ppm.tile([16, 4], f32, name="p_h")
                for hi in range(2):
                    nc.tensor.matmul(
                        p_h[:],
                        w1_t[:, hi * 16 : (hi + 1) * 16],
                        sT_all[:, (2 * b + hi) * 4 : (2 * b + hi + 1) * 4],
                        start=(hi == 0),
                        stop=(hi == 1),
                    )
                h_sb = mpool.tile([16, 4], f32, name=f"h{b}")
                nc.scalar.activation(h_sb[:], p_h[:], AF.Relu, bias=b1_t)
                hs.append(h_sb)
            # scores in hi-major column order so the b2 bias is a plain
            # per-partition AP fused into the exp activation
            p_sc = ppm.tile([128, 16], f32, name="p_sc")
            for b in range(BPC):
                for hi in range(2):
                    g = 2 * hi + b
                    nc.tensor.matmul(
                        p_sc[:, 4 * g : 4 * g + 4],
                        w2_t[:, hi * 128 : (hi + 1) * 128],
                        hs[b][:],
                        start=True, stop=True,
                    )
            # no max-subtraction: scores are mean-pooled SE-MLP outputs with
            # |score| << 80, so exp cannot overflow in f32
            e_all = mpool.tile([128, 4, 4], f32, name="e_all")
            for hi in range(2):
                nc.scalar.activation(
                    e_all[:].rearrange("p a r -> p (a r)")[:, hi * 8 : (hi + 1) * 8],
                    p_sc[:, hi * 8 : (hi + 1) * 8],
                    AF.Exp,
                    bias=b2_t[:, hi : hi + 1],
                )
            sm = mpool.tile([128, 4], f32, name="sm")
            nc.vector.reduce_sum(sm[:], e_all[:], axis=AX.X)
            rcp = mpool.tile([128, 4], f32, name="rcp")
            nc.vector.reciprocal(rcp[:], sm[:])
            rc = rcp[:]
            rcv = bass.AP(rc.tensor, rc.offset, [rc.ap[0], rc.ap[1], [0, 4]])
            W_all = mpool.tile([128, 4, 4], f32, name="W_all")
            nc.vector.tensor_tensor(W_all[:], e_all[:], rcv, op=ALU.mult)

            # diag(W[r, j*128+p]) tiles for the PE weighted sum; W_all group
            # g = 2*hi + b for j = 2*b + hi; split across DVE and ACT
            for r in range(4):
                for j in range(4):
                    g = 2 * (j % 2) + j // 2
                    dst = dg16[:, (r * 4 + j) * 128 : (r * 4 + j + 1) * 128]
                    if (r * 4 + j) % 2 == 0:
                        nc.vector.tensor_scalar_mul(
                            dst, ident16, W_all[:, g, r : r + 1]
                        )
                    else:
                        nc.scalar.mul(dst, ident16, W_all[:, g, r : r + 1])

        # deferred x16 casts for the final columns (needed only by the last
        # phase-2 chunks), emitted here so they queue behind the MLP ops
        for par, cast_src, cast_dst in late_casts:
            if par == 0:
                nc.vector.tensor_copy(cast_dst, cast_src)
            else:
                nc.scalar.copy(cast_dst, cast_src)

        # ---------------- phase 2: gather fp16, PE-weighted sum, store ------
        gathers, pchunks, gbufs = _plan(skip_r0)
        with tc.tile_pool(name="pp2", bufs=8, space="PSUM") as pp2:
            gmap = []          # (col0, width, gather-output tile)
            gi = 0
            gcol = 0
            par = 0
            for c0, w in pchunks:
                while gi < len(gathers) and gathers[gi][0] <= c0:
                    g0, gw = gathers[gi]
                    gn = NRG * gw
                    gt = gpool.tile(
                        [128, 4, gn], f16, name=f"gt{gi}",
                        tag=f"g{gw}", bufs=gbufs[gw],
                    )
                    nc.gpsimd.dma_gather(
                        gt[:],
                        xt[:],
                        idx_t[:, gcol : gcol + gn // 16],
                        gn,
                        gn,
                        BC,
                        transpose=True,
                        sbuf_tokens_per_rank=128,
                        sbuf_free_dim_per_rank=BC * 2,
                    )
                    gmap.append((g0, gw, gt))
                    gcol += gn // 16
                    gi += 1
                g0, gw, gt = next(g for g in reversed(gmap) if g[0] <= c0)
                off = c0 - g0
                ot = opool.tile([128, 4, NCH], f16, name="ot")
                # j-pair PSUM tiles (1 bank each, 8 bufs) so drains recycle
                # PSUM at half-chunk granularity and PE never backlogs
                for jp in range(2):
                    po = pp2.tile([128, 2, NCH], f32, name="po")
                    for jj in range(2):
                        j = 2 * jp + jj
                        for r in range(4):
                            if skip_r0 and r == 0:
                                rhs = x16[:, j, c0 : c0 + w]
                            else:
                                ri = r - 1 if skip_r0 else r
                                rhs = gt[:, j, ri * gw + off : ri * gw + off + w]
                            nc.tensor.matmul(
                                po[:, jj, :w],
                                dg16[:, (r * 4 + j) * 128 : (r * 4 + j + 1) * 128],
                                rhs,
                                start=(r == 0),
                                stop=(r == 3),
                                skip_group_check=True,
                            )
                    dst = ot[:, 2 * jp : 2 * jp + 2, :w]
                    if par % 2 == 0:
                        nc.scalar.copy(dst, po[:, :, :w])
                    else:
                        nc.vector.tensor_copy(dst, po[:, :, :w])
                    par += 1
                nc.sync.dma_start(out3[:, :, c0 : c0 + w], ot[:, :, :w])

    nc.compile()
    return nc


def _host_prep(x, rot_idx, w1, b1, w2, b2):
    x = np.asarray(x, dtype=np.float32)
    rot_idx = np.asarray(rot_idx, dtype=np.int64)
    w1 = np.asarray(w1, dtype=np.float32)
    b1 = np.asarray(b1, dtype=np.float32)
    w2 = np.asarray(w2, dtype=np.float32)
    b2 = np.asarray(b2, dtype=np.float32)

    skip_r0 = bool(np.array_equal(rot_idx[0], np.arange(L)))
    NRG = 3 if skip_r0 else 4

    cnt = np.zeros((R, L), dtype=np.float32)
    for r in range(R):
        cnt[r] = np.bincount(rot_idx[r], minlength=L).astype(np.float32)
    cnt /= np.float32(L)
    # cnt16[p, 4t+r] = cnt[r, t*128+p] (counts/L are exact in fp16)
    cnt_sb = np.ascontiguousarray(
        cnt.T.reshape(NT, 128, R).transpose(1, 0, 2).reshape(128, 128)
    )

    cst = np.zeros((128, CST_W), dtype=np.float32)
    cst[:, CST_ID : CST_ID + 128] = np.eye(128, dtype=np.float32)
    cst[:, CST_W1 : CST_W1 + 32] = (
        w1.reshape(2, 128, RED).transpose(1, 0, 2).reshape(128, 2 * RED)
    )
    cst[:, CST_B2 : CST_B2 + 2] = b2.reshape(2, 128).T
    cst[0:16, CST_W2 : CST_W2 + 256] = w2
    cst[0:16, CST_B1] = b1

    cst16 = np.zeros((128, C16_W), dtype=np.float16)
    cst16[:, C16_ID : C16_ID + 128] = np.eye(128, dtype=np.float16)
    cst16[:, C16_CNT : C16_CNT + 128] = cnt_sb.astype(np.float16)

    # gather index table: per gather call (col0, gw), linear order
    # [r1 l's, r2 l's, r3 l's] (plus r0 first when not skipped), wrapped
    # idx[p, s] = lin[s*16 + p], replicated over 8 groups of 16 partitions
    idx_sb = np.zeros((128, NRG * L // 16), dtype=np.int16)
    rlist = range(1, R) if skip_r0 else range(R)
    gcol = 0
    gathers, _, _ = _plan(skip_r0)
    for g0, gw in gathers:
        gc = NRG * gw // 16
        lin = np.concatenate(
            [rot_idx[r, g0 : g0 + gw] for r in rlist]
        ).astype(np.int16)
        block = lin.reshape(gc, 16).T  # [16, gc]
        idx_sb[:, gcol : gcol + gc] = np.tile(block, (8, 1))
        gcol += gc

    shared = {"cst": cst, "cst16": cst16, "idx": idx_sb}
    in_maps = []
    for c in range(NCORES):
        mm = dict(shared)
        mm["x"] = np.ascontiguousarray(x[c * BPC : (c + 1) * BPC].reshape(BC, L))
        in_maps.append(mm)
    return skip_r0, in_maps


def kernel(x, rot_idx, w1, b1, w2, b2, _trace=False):
    from concourse import bass_utils

    skip_r0, in_maps = _host_prep(x, rot_idx, w1, b1, w2, b2)
    key = ("nc", skip_r0)
    if key not in _NC_CACHE:
        _NC_CACHE[key] = _build_nc(skip_r0)
    nc = _NC_CACHE[key]
    _NC_CACHE["nc"] = nc  # for test harness TimelineSim access
    res = bass_utils.run_bass_kernel_spmd(
        nc, in_maps, core_ids=list(range(NCORES)), trace=_trace
    )
    out = np.empty((B, C, L), dtype=np.float32)
    for c in range(NCORES):
        # device stores fp16 (within tolerance); widen during the unshard
        out[c * BPC : (c + 1) * BPC] = (
            res.results[c]["out"].astype(np.float32).reshape(BPC, C, L)
        )
    if _trace:
        kernel.last_results = res
    return out


# revision 53
# speedup vs baseline: 1.1215x; 1.0001x over previous
"""Trainium2 Bass kernel for CyclicShiftConv (Hilbert-rotation SE attention).

out[b,c,l] = sum_r softmax_r(MLP(mean_l x[b,c,rot_idx[r,l]]))[b,c,r] * x[b,c,rot_idx[r,l]]

Strategy (8 cores, data-parallel over batch; 2 samples = 512 (b,c) rows/core).
The kernel is DMA-bound (360 GB/s shared DMA fabric), so the design minimizes
bytes moved:
  - x (8 MiB f32/core) is loaded once, PE-transposed into an fp16 x^T resident
    in SBUF (no DRAM round-trip for the transposed copy).
  - mean over l of the gathered tensor == x^T contracted with bincount/L, so
    the SE-MLP input comes from tiny PE matmuls against an fp16 count vector.
  - The rotation gathers run SBUF->SBUF with the fp16 transpose-mode
    dma_gather (tokens_per_rank=128 maps token l' to partition l'%128 / rank
    l'//128, exactly the x^T stripe layout).  Transpose-mode writes return the
    data in natural [bc, l] layout, so no transpose-back matmuls are needed.
    fp16 halves gather bytes vs f32.
  - rot_idx[0] is the identity for the Hilbert rotation set; when the host
    detects this, rotation 0 is not gathered at all - its contribution comes
    from a natural-layout fp16 copy of x cast during phase 1 (4 MiB less DMA).
    Arbitrary (non-identity) index tables fall back to gathering all four
    rotations with 128-wide gather calls (512 descriptors each - a 1024-desc
    call would fill the SWDGE ring carveout and hang).
  - The softmax-weighted sum over rotations runs on the PE as matmuls against
    diagonal weight matrices accumulating in PSUM (out_j = sum_r
    diag(W_r) @ g_r), keeping the elementwise engines free for PSUM drains.
  - The output is stored fp16 on device (error ~3e-4 << the 2e-2 tolerance)
    and widened to f32 during the host-side unshard, halving output DMA.
"""

import sys

for _p in ("/opt/trn_rl_repo", "/opt/pypackages"):
    if _p not in sys.path:
        sys.path.append(_p)

import numpy as np

B, C, L, R, RED = 16, 256, 4096, 4, 16
NCORES = 8
BPC = B // NCORES          # samples per core
BC = BPC * C               # 512 rows per core
NT = L // 128              # 32 transpose stripes
NCH = 256                  # phase-2 max l-positions per processing chunk
# gather calls (col0, width): big early for DMA density / low desc-gen
# overhead, small late so the post-last-gather tail is short
GATHERS = [(i * 256, 256) for i in range(16)]
# processing chunks (col0, width): PE/drain/store granularity
PCHUNKS = [(i * 256, 256) for i in range(16)]
GBUFS = {256: 7}   # gather-buffer ring depth per width


def _plan(skip_r0):
    """Gather/processing chunking per variant. The 4-rotation fallback uses
    128-wide gathers: 4*256=1024 descriptors per call hits the SWDGE ring
    carveout (16384/16) and hangs the device; 4*128=512 is safe."""
    if skip_r0:
        return GATHERS, PCHUNKS, GBUFS
    g = [(i * 128, 128) for i in range(32)]
    return g, g, {128: 8}

# f32 constant block: [128, CST_W]
CST_ID = 0                 # identity [128, 128]
CST_W1 = 128               # w1       [128, 32]
CST_B2 = 160               # b2       [128, 2]
CST_W2 = 162               # w2       [16, 256] (partitions 0:16)
CST_B1 = 418               # b1       [16, 1]
CST_W = 419

# fp16 constant block: [128, 256]
C16_ID = 0                 # identity [128, 128]
C16_CNT = 128              # cnt      [128, 128]: cnt16[p, 4t+r] = cnt[r, t*128+p]
C16_W = 256

_NC_CACHE = {}


def _build_nc(skip_r0, loop_n=1):
    import concourse.bass as bass
    import concourse.mybir as mybir
    from concourse import bacc
    from concourse.tile import TileContext
    from contextlib import ExitStack

    f32 = mybir.dt.float32
    f16 = mybir.dt.float16
    i16 = mybir.dt.int16
    AF = mybir.ActivationFunctionType
    AX = mybir.AxisListType
    ALU = mybir.AluOpType

    NRG = 3 if skip_r0 else 4      # rotations gathered
    NIDXC = NRG * L // 16          # idx table columns

    f32r = mybir.dt.float32r

    nc = bacc.Bacc(
        "TRN2",
        target_bir_lowering=False,
        debug=False,
        enable_asserts=False,
        num_devices=NCORES,
    )

    x_in = nc.dram_tensor("x", [BC, L], f32, kind="ExternalInput").ap()
    cst_in = nc.dram_tensor("cst", [128, CST_W], f32, kind="ExternalInput").ap()
    cst16_in = nc.dram_tensor("cst16", [128, C16_W], f16, kind="ExternalInput").ap()
    idx_in = nc.dram_tensor("idx", [128, NIDXC], i16, kind="ExternalInput").ap()
    out = nc.dram_tensor("out", [BC, L], f16, kind="ExternalOutput").ap()

    x3 = x_in.rearrange("(j p) l -> p j l", j=4)
    out3 = out.rearrange("(j p) l -> p j l", j=4)

    with TileContext(nc) as tc, ExitStack() as ctx:
        if loop_n > 1:
            ctx.enter_context(tc.For_i(0, loop_n, 1))
        cpool = ctx.enter_context(tc.tile_pool(name="consts", bufs=1))
        xpool = ctx.enter_context(tc.tile_pool(name="xp", bufs=1))
        # phase-2 pools opened up-front so their SBUF ranges never overlap
        # the phase-1/MLP pools (address reuse creates false WAR waits)
        gpool = ctx.enter_context(tc.tile_pool(name="gp", bufs=6))
        opool = ctx.enter_context(tc.tile_pool(name="op", bufs=6))

        cst = cpool.tile([128, CST_W], f32, name="cst")
        nc.sync.dma_start(cst[:], cst_in)
        cst16 = cpool.tile([128, C16_W], f16, name="cst16")
        idx_t = cpool.tile([128, NIDXC], i16, name="idx_t")

        ident = cst[:, CST_ID : CST_ID + 128]
        w1_t = cst[:, CST_W1 : CST_W1 + 32]
        b2_t = cst[:, CST_B2 : CST_B2 + 2]
        w2_t = cst[0:16, CST_W2 : CST_W2 + 256]
        b1_t = cst[0:16, CST_B1 : CST_B1 + 1]
        ident16 = cst16[:, C16_ID : C16_ID + 128]
        cnt16 = cst16[:, C16_CNT : C16_CNT + 128]

        # x natural fp16: [128, j, L], bc = j*128 + p (kept through phase 2)
        x16 = xpool.tile([128, 4, L], f16, name="x16")
        # x^T fp16: xt[p, t*512 + k] = x[bc=k, l=t*128+p]
        xt = xpool.tile([128, NT * BC], f16, name="xt")
        s_sb = cpool.tile([4, BC], f32, name="s_sb")
        # diagonal softmax-weight matrices for the PE weighted sum
        dg16 = cpool.tile([128, 4 * 4 * 128], f16, name="dg16")

        # ---------------- phase 1: load, transpose -> xt fp16, s = x @ cnt --
        # late-half x16 casts deferred past the MLP: (engine-parity, src AP,
        # dst AP) emitted after the softmax so the cast queue never delays it
        late_casts = []
        with (
            tc.tile_pool(name="xsp", bufs=6) as xspool,
            tc.tile_pool(name="pp1", bufs=6, space="PSUM") as pp1,
            tc.tile_pool(name="pps", bufs=1, space="PSUM") as pps,
        ):
            # shrinking load chunks: the final small chunks minimize the
            # last-stripe transpose+drain latency that gates the first gather
            widths = [512] * 6 + [256] * 3 + [128, 128]
            col = 0
            tglob = 0
            for h, w in enumerate(widths):
                if h >= 9:
                    # final chunks live in the persistent pool: their x16
                    # casts are deferred past the MLP (see late_casts)
                    xh = xpool.tile([128, 4, w], f32, name=f"xh_l{h}")
                else:
                    xh = xspool.tile([128, 4, w], f32, name="xh")
                if h == 0:
                    # first load issued from ACT's DGE so its descriptor prep
                    # overlaps SP's cst issue instead of queueing behind it
                    nc.scalar.dma_start(xh[:], x3[:, :, col : col + w])
                    # cnt16 feeds the interleaved s-matmuls: keep it early
                    nc.sync.dma_start(cst16[:], cst16_in)
                else:
                    nc.sync.dma_start(xh[:], x3[:, :, col : col + w])
                if h == len(widths) - 1:
                    # idx is only read by gather desc-gen (~30us); loading it
                    # last shaves its slot off the x-load critical path
                    nc.sync.dma_start(idx_t[:], idx_in)
                for tt in range(w // 128):
                    t = tglob + tt
                    pt = pp1.tile([128, BC], f32, name="pt")
                    for j in range(4):
                        nc.tensor.transpose(
                            pt[:, j * 128 : (j + 1) * 128],
                            xh[:, j, tt * 128 : (tt + 1) * 128],
                            ident,
                        )
                    xt_t = xt[:, t * BC : (t + 1) * BC]
                    if t % 2 == 0:
                        nc.vector.tensor_copy(xt_t, pt[:])
                    else:
                        nc.scalar.copy(xt_t, pt[:])
                # natural fp16 copy (consumed by phase 2's rotation-0 term);
                # NCH-aligned pieces, emitted after the drains so they never
                # head-of-line-block an xt drain in the engine queues
                pw = min(w, NCH)
                for piece in range(w // pw):
                    cast_src = xh[:, :, piece * pw : (piece + 1) * pw]
                    cast_dst = x16[:, :, col + piece * pw : col + (piece + 1) * pw]
                    if h < 7:
                        if (h + piece) % 2 == 0:
                            nc.vector.tensor_copy(cast_dst, cast_src)
                        else:
                            nc.scalar.copy(cast_dst, cast_src)
                    else:
                        late_casts.append(((h + piece) % 2, cast_src, cast_dst))
                col += w
                tglob += w // 128
            # batched mean matmuls (decoupled from the per-stripe chain so the
            # PE never stalls on a drain mid-phase)
            psum_s = pps.tile([4, BC], f32, name="psum_s")
            for t in range(NT):
                nc.tensor.matmul(
                    psum_s[:],
                    cnt16[:, 4 * t : 4 * t + 4],
                    xt[:, t * BC : (t + 1) * BC],
                    start=(t == 0),
                    stop=(t == NT - 1),
                )
            nc.vector.tensor_copy(s_sb[:], psum_s[:])

        # ---------------- SE MLP + softmax over rotations -------------------
        with (
            tc.tile_pool(name="mlp", bufs=1) as mpool,
            tc.tile_pool(name="ppm", bufs=1, space="PSUM") as ppm,
        ):
            # one PSUM tile + single drain for all 4 s^T transposes
            p_sT4 = ppm.tile([128, 16], f32, name="p_sT4")
            for j in range(4):
                nc.tensor.transpose(
                    p_sT4[:, 4 * j : 4 * j + 4],
                    s_sb[:, j * 128 : (j + 1) * 128],
                    cst[0:4, CST_ID : CST_ID + 4],
                )
            sT_all = mpool.tile([128, 16], f32, name="sT_all")
            nc.vector.tensor_copy(sT_all[:], p_sT4[:])
            hs = []
            for b in range(BPC):
                p_h = ppm.tile([16, 4], f32, name="p_h")
                for hi in range(2):
                    nc.tensor.matmul(
                        p_h[:],
                        w1_t[:, hi * 16 : (hi + 1) * 16],
                        sT_all[:, (2 * b + hi) * 4 : (2 * b + hi + 1) * 4],
                        start=(hi == 0),
                        stop=(hi == 1),
                    )
                h_sb = mpool.tile([16, 4], f32, name=f"h{b}")
                nc.scalar.activation(h_sb[:], p_h[:], AF.Relu, bias=b1_t)
                hs.append(h_sb)
            # scores in hi-major column order so the b2 bias is a plain
            # per-partition AP fused into the exp activation
            p_sc = ppm.tile([128, 16], f32, name="p_sc")
            for b in range(BPC):
                for hi in range(2):
                    g = 2 * hi + b
                    nc.tensor.matmul(
                        p_sc[:, 4 * g : 4 * g + 4],
                        w2_t[:, hi * 128 : (hi + 1) * 128],
                        hs[b][:],
                        start=True, stop=True,
                    )
            # no max-subtraction: scores are mean-pooled SE-MLP outputs with
            # |score| << 80, so exp cannot overflow in f32
            e_all = mpool.tile([128, 4, 4], f32, name="e_all")
            for hi in range(2):
                nc.scalar.activation(
                    e_all[:].rearrange("p a r -> p (a r)")[:, hi * 8 : (hi + 1) * 8],
                    p_sc[:, hi * 8 : (hi + 1) * 8],
                    AF.Exp,
                    bias=b2_t[:, hi : hi + 1],
                )
            sm = mpool.tile([128, 4], f32, name="sm")
            nc.vector.reduce_sum(sm[:], e_all[:], axis=AX.X)
            rcp = mpool.tile([128, 4], f32, name="rcp")
            nc.vector.reciprocal(rcp[:], sm[:])
            rc = rcp[:]
            rcv = bass.AP(rc.tensor, rc.offset, [rc.ap[0], rc.ap[1], [0, 4]])
            W_all = mpool.tile([128, 4, 4], f32, name="W_all")
            nc.vector.tensor_tensor(W_all[:], e_all[:], rcv, op=ALU.mult)

            # diag(W[r, j*128+p]) tiles for the PE weighted sum; W_all group
            # g = 2*hi + b for j = 2*b + hi; split across DVE and ACT
            for r in range(4):
                for j in range(4):
                    g = 2 * (j % 2) + j // 2
                    dst = dg16[:, (r * 4 + j) * 128 : (r * 4 + j + 1) * 128]
                    if (r * 4 + j) % 2 == 0:
                        nc.vector.tensor_scalar_mul(
                            dst, ident16, W_all[:, g, r : r + 1]
                        )
                    else:
                        nc.scalar.mul(dst, ident16, W_all[:, g, r : r + 1])

        # deferred x16 casts for the final columns (needed only by the last
        # phase-2 chunks), emitted here so they queue behind the MLP ops
        for par, cast_src, cast_dst in late_casts:
            if par == 0:
                nc.vector.tensor_copy(cast_dst, cast_src)
            else:
                nc.scalar.copy(cast_dst, cast_src)

        # ---------------- phase 2: gather fp16, PE-weighted sum, store ------
        gathers, pchunks, gbufs = _plan(skip_r0)
        with tc.tile_pool(name="pp2", bufs=8, space="PSUM") as pp2:
            gmap = []          # (col0, width, gather-output tile)
            gi = 0
            gcol = 0
            par = 0
            for c0, w in pchunks:
                while gi < len(gathers) and gathers[gi][0] <= c0:
                    g0, gw = gathers[gi]
                    gn = NRG * gw
                    gt = gpool.tile(
                        [128, 4, gn], f16, name=f"gt{gi}",
                        tag=f"g{gw}", bufs=gbufs[gw],
                    )
                    nc.gpsimd.dma_gather(
                        gt[:],
                        xt[:],
                        idx_t[:, gcol : gcol + gn // 16],
                        gn,
                        gn,
                        BC,
                        transpose=True,
                        sbuf_tokens_per_rank=128,
                        sbuf_free_dim_per_rank=BC * 2,
                    )
                    gmap.append((g0, gw, gt))
                    gcol += gn // 16
                    gi += 1
                g0, gw, gt = next(g for g in reversed(gmap) if g[0] <= c0)
                off = c0 - g0
                ot = opool.tile([128, 4, NCH], f16, name="ot")
                # j-pair PSUM tiles (1 bank each, 8 bufs) so drains recycle
                # PSUM at half-chunk granularity and PE never backlogs
                for jp in range(2):
                    po = pp2.tile([128, 2, NCH], f32, name="po")
                    for jj in range(2):
                        j = 2 * jp + jj
                        for r in range(4):
                            if skip_r0 and r == 0:
                                rhs = x16[:, j, c0 : c0 + w]
                            else:
                                ri = r - 1 if skip_r0 else r
                                rhs = gt[:, j, ri * gw + off : ri * gw + off + w]
                            nc.tensor.matmul(
                                po[:, jj, :w],
                                dg16[:, (r * 4 + j) * 128 : (r * 4 + j + 1) * 128],
                                rhs,
                                start=(r == 0),
                                stop=(r == 3),
                                skip_group_check=True,
                            )
                    dst = ot[:, 2 * jp : 2 * jp + 2, :w]
                    if par % 2 == 0:
                        nc.scalar.copy(dst, po[:, :, :w])
                    else:
                        nc.vector.tensor_copy(dst, po[:, :, :w])
                    par += 1
                nc.sync.dma_start(out3[:, :, c0 : c0 + w], ot[:, :, :w])

    nc.compile()
    return nc


def _host_prep(x, rot_idx, w1, b1, w2, b2):
    x = np.asarray(x, dtype=np.float32)
    rot_idx = np.asarray(rot_idx, dtype=np.int64)
    w1 = np.asarray(w1, dtype=np.float32)
    b1 = np.asarray(b1, dtype=np.float32)
    w2 = np.asarray(w2, dtype=np.float32)
    b2 = np.asarray(b2, dtype=np.float32)

    skip_r0 = bool(np.array_equal(rot_idx[0], np.arange(L)))
    NRG = 3 if skip_r0 else 4

    cnt = np.zeros((R, L), dtype=np.float32)
    for r in range(R):
        cnt[r] = np.bincount(rot_idx[r], minlength=L).astype(np.float32)
    cnt /= np.float32(L)
    # cnt16[p, 4t+r] = cnt[r, t*128+p] (counts/L are exact in fp16)
    cnt_sb = np.ascontiguousarray(
        cnt.T.reshape(NT, 128, R).transpose(1, 0, 2).reshape(128, 128)
    )

    cst = np.zeros((128, CST_W), dtype=np.float32)
    cst[:, CST_ID : CST_ID + 128] = np.eye(128, dtype=np.float32)
    cst[:, CST_W1 : CST_W1 + 32] = (
        w1.reshape(2, 128, RED).transpose(1, 0, 2).reshape(128, 2 * RED)
    )
    cst[:, CST_B2 : CST_B2 + 2] = b2.reshape(2, 128).T
    cst[0:16, CST_W2 : CST_W2 + 256] = w2
    cst[0:16, CST_B1] = b1

    cst16 = np.zeros((128, C16_W), dtype=np.float16)
    cst16[:, C16_ID : C16_ID + 128] = np.eye(128, dtype=np.float16)
    cst16[:, C16_CNT : C16_CNT + 128] = cnt_sb.astype(np.float16)

    # gather index table: per gather call (col0, gw), linear order
    # [r1 l's, r2 l's, r3 l's] (plus r0 first when not skipped), wrapped
    # idx[p, s] = lin[s*16 + p], replicated over 8 groups of 16 partitions
    idx_sb = np.zeros((128, NRG * L // 16), dtype=np.int16)
    rlist = range(1, R) if skip_r0 else range(R)
    gcol = 0
    gathers, _, _ = _plan(skip_r0)
    for g0, gw in gathers:
        gc = NRG * gw // 16
        lin = np.concatenate(
            [rot_idx[r, g0 : g0 + gw] for r in rlist]
        ).astype(np.int16)
        block = lin.reshape(gc, 16).T  # [16, gc]
        idx_sb[:, gcol : gcol + gc] = np.tile(block, (8, 1))
        gcol += gc

    shared = {"cst": cst, "cst16": cst16, "idx": idx_sb}
    in_maps = []
    for c in range(NCORES):
        mm = dict(shared)
        mm["x"] = np.ascontiguousarray(x[c * BPC : (c + 1) * BPC].reshape(BC, L))
        in_maps.append(mm)
    return skip_r0, in_maps


def kernel(x, rot_idx, w1, b1, w2, b2, _trace=False):
    from concourse import bass_utils

    skip_r0, in_maps = _host_prep(x, rot_idx, w1, b1, w2, b2)
    key = ("nc", skip_r0)
    if key not in _NC_CACHE:
        _NC_CACHE[key] = _build_nc(skip_r0)
    nc = _NC_CACHE[key]
    _NC_CACHE["nc"] = nc  # for test harness TimelineSim access
    res = bass_utils.run_bass_kernel_spmd(
        nc, in_maps, core_ids=list(range(NCORES)), trace=_trace
    )
    out = np.empty((B, C, L), dtype=np.float32)
    for c in range(NCORES):
        # device stores fp16 (within tolerance); widen during the unshard
        out[c * BPC : (c + 1) * BPC] = (
            res.results[c]["out"].astype(np.float32).reshape(BPC, C, L)
        )
    if _trace:
        kernel.last_results = res
    return out
